# revision 1
# baseline (speedup 1.0000x reference)
"""DiffFDN Trainium2 kernel, v3: DRAM history + indirect gathers.

Per core (4 items): the 48000-step FDN scan becomes 94 blocks of
[64x68]^T @ [64x500] (float32r). History lives in DRAM as one
[68, TPAD] tensor (rows 0-63 per-(line,item) nxt series, rows 64-67 the
y output series). Per block: one PSUM->SBUF copy, one staged HWDGE
write to DRAM, one SWDGE *indirect* gather (per-row flat offsets) that
assembles the 16 time-shifted line reads in a single instruction.

The gather's in_ AP is the column-sliced prefix H[:, 0:PAD+n0-500] so
the Tile dependency tracker sees the true dependency (the write from
block b-2), keeping a 2-deep software pipeline; offsets are view-flat
element indices r*X_b + (PAD + n0 - d_i).
"""

import numpy as np

SR = 48000
IR_LEN = 48000
DELAYS = [1009, 1123, 1231, 1321, 1433, 1543, 1657, 1777, 1879, 1987,
          2081, 2179, 2287, 2383, 2503, 2617]
N = 16
FEAT = 256
BATCH = 32
NCORES = 8
IPC = BATCH // NCORES
L = 500
PAD = 2620                     # zero padding before t=0 (>= max delay)
TPAD = PAD + IR_LEN + 500
NBLK = IR_LEN // L             # 96; blocks 0,1 skipped (all-zero)
M_OUT = IPC * N + IPC          # 68

_BUILT = None
K_DEP = 2          # gather only covers blocks <= b-K_DEP (pipeline depth)


def _patch_list():
    """Pieces of each line's read window sourced from blocks > b-K_DEP.

    These are patched into S from the SBUF stage tiles (the DRAM gather
    raced/skipped those regions). Returns (line, rel_blk, src_col,
    dst_col, length) tuples; rel_blk is source block minus b.
    """
    out = []
    for i in range(N):
        d = DELAYS[i]
        lo, hi = -d, -d + L
        a = lo
        while a < hi:
            e = min(hi, (a // L + 1) * L)
            rel = a // L
            if rel >= -(K_DEP - 1):
                out.append((i, rel, a - rel * L, a - lo, e - a))
            a = e
    return out


def _expm64(M):
    M = M.astype(np.float64)
    nrm = np.linalg.norm(M, ord=np.inf)
    k = max(0, int(np.ceil(np.log2(max(nrm, 1e-30)))) + 2)
    Ms = M / (2.0 ** k)
    E = np.eye(M.shape[0]) + Ms
    term = Ms.copy()
    for i in range(2, 18):
        term = term @ Ms / i
        E = E + term
    for _ in range(k):
        E = E @ E
    return E


def _prologue(x, WA, bA, WB, bB, WC, bC):
    x = np.asarray(x, np.float32)
    feat = x.mean(axis=1)
    A = np.tanh(feat @ np.asarray(WA).T + bA).reshape(-1, N, N)
    Bv = np.tanh(feat @ np.asarray(WB).T + bB)
    Cv = np.tanh(feat @ np.asarray(WC).T + bC)
    S = np.triu(A, 1)
    S = S - np.swapaxes(S, -1, -2)
    g = 10.0 ** (-3.0 / SR)
    G = g ** np.asarray(DELAYS, np.float64)
    A_g = np.stack([_expm64(S[b]) for b in range(S.shape[0])])
    A_g = (A_g * G[None, None, :]).astype(np.float32)
    return A_g, Bv.astype(np.float32), Cv.astype(np.float32)


def _core_inputs(A_g4, Bv4, Cv4):
    lhsT = np.zeros((IPC * N, M_OUT), np.float32)
    bv = np.zeros((IPC * N, 1), np.float32)
    for j in range(IPC):
        for i in range(N):
            r = 4 * i + j
            for ip in range(N):
                lhsT[r, 4 * ip + j] = A_g4[j, ip, i]
            lhsT[r, IPC * N + j] = Cv4[j, i]
            bv[r, 0] = Bv4[j, i]
    return lhsT, bv


OFFS_PHYSICAL = True


def _offsets():
    """offs[r, b-2] = flat gather offset for row r, sub-block b.

    OFFS_PHYSICAL: offsets are element offsets into the physical tensor
    (row stride TPAD) -- what the HW descriptor generator uses. CoreSim
    instead flattens the sliced view (row stride X_b).
    """
    offs = np.zeros((IPC * N, NBLK - 2), np.uint32)
    for b in range(2, NBLK):
        n0 = L * b
        Xb = TPAD if OFFS_PHYSICAL else (PAD + n0 - L)
        for i in range(N):
            for j in range(IPC):
                r = 4 * i + j
                offs[r, b - 2] = r * Xb + (PAD + n0 - DELAYS[i])
    return offs


def _build():
    global _BUILT
    if _BUILT is not None:
        return _BUILT
    import concourse.bacc as bacc
    import concourse.bass as bass
    import concourse.mybir as mybir
    import concourse.tile as tile

    fp32 = mybir.dt.float32
    f32r = mybir.dt.float32r
    u32 = mybir.dt.uint32
    nc = bacc.Bacc("TRN2", target_bir_lowering=False, debug=False)
    lhsT_d = nc.dram_tensor("lhsT", [IPC * N, M_OUT], f32r, kind="ExternalInput")
    bv_d = nc.dram_tensor("bv", [IPC * N, 1], f32r, kind="ExternalInput")
    offs_d = nc.dram_tensor("offs", [IPC * N, NBLK - 2], u32, kind="ExternalInput")
    patches = _patch_list()
    npat = len(patches)
    pmask_d = None
    if npat:
        pmask_d = nc.dram_tensor(
            "pmask", [IPC * N, npat], mybir.dt.uint8, kind="ExternalInput")
    y_d = nc.dram_tensor("y", [IPC, IR_LEN], f32r, kind="ExternalOutput")
    h_d = nc.dram_tensor("hist", [M_OUT, TPAD], f32r)

    with tile.TileContext(nc) as tc:
        with tc.tile_pool(name="const", bufs=1) as cpool, \
             tc.tile_pool(name="init", bufs=1) as ipool, \
             tc.tile_pool(name="sg", bufs=8) as spool, \
             tc.tile_pool(name="st", bufs=10) as tpool, \
             tc.tile_pool(name="ps", bufs=8, space="PSUM") as ppool, \
             tc.tile_pool(name="yb", bufs=2) as ypool:
            lhsT = cpool.tile([IPC * N, M_OUT], f32r)
            nc.sync.dma_start(lhsT[:, :], lhsT_d[:, :])
            offs = cpool.tile([IPC * N, NBLK - 2], u32)
            nc.sync.dma_start(offs[:, :], offs_d[:, :])
            pmask = None
            if npat:
                pmask = cpool.tile([IPC * N, npat], mybir.dt.uint8)
                nc.sync.dma_start(pmask[:, :], pmask_d[:, :])

            # zero-init history cols [0, PAD+1000) incl. y rows; Bv impulse
            # lands at col PAD (time 0) via the same staged image.
            z = ipool.tile([M_OUT, PAD + 2 * L], fp32)
            half = (PAD + 2 * L) // 2
            nc.vector.memset(z[:, 0:half], 0.0)
            nc.gpsimd.memset(z[:, half:], 0.0)
            nc.sync.dma_start(z[0:IPC * N, PAD:PAD + 1].bitcast(f32r), bv_d[:, :])
            nc.scalar.dma_start(
                h_d[:, 0:PAD + 2 * L].bitcast(fp32), z[:, :])

            stages = {}  # b -> (tile, col0) holding that block's nxt in SBUF
            stages[-1] = (z, PAD - L)   # negative time: zeros
            stages[0] = (z, PAD)
            stages[1] = (z, PAD + L)
            for b in range(2, NBLK):
                n0 = L * b
                Xb = PAD + n0 - (K_DEP - 1) * L
                S = spool.tile([IPC * N, L], f32r)
                nc.gpsimd.indirect_dma_start(
                    out=S[:, :], out_offset=None,
                    in_=h_d[0:IPC * N, 0:Xb],
                    in_offset=bass.IndirectOffsetOnAxis(
                        ap=offs[:, b - 2:b - 1], axis=1),
                )
                # patch recent-sourced pieces of S from the SBUF stages
                # (the DRAM gather raced/skipped those regions). Engines
                # require 32-aligned partition bases, so each patch runs
                # base-0 over all rows with a per-line row mask.
                for k, (i, rel, sc, dc, ln) in enumerate(patches):
                    ptile, pcol = stages[b + rel]
                    src = ptile[0:IPC * N, pcol + sc:pcol + sc + ln]
                    if src.dtype != f32r:
                        src = src.bitcast(f32r)
                    nc.vector.copy_predicated(
                        S[:, dc:dc + ln],
                        pmask[:, k:k + 1].to_broadcast([IPC * N, ln]),
                        src,
                    )
                ps = ppool.tile([M_OUT, L], fp32)
                nc.tensor.matmul(ps[:, :], lhsT[:, :], S[:, :],
                                 start=True, stop=True)
                stage = tpool.tile([M_OUT, L], f32r)
                stages[b] = (stage, 0)
                if b % 2 == 0:
                    nc.vector.tensor_copy(stage[:, :], ps[:, :])
                else:
                    nc.scalar.copy(stage[:, :], ps[:, :])
                weng = nc.sync if b % 2 == 0 else nc.scalar
                weng.dma_start(h_d[:, PAD + n0:PAD + n0 + L], stage[:, :])

                # y extraction (hist rows 64..67 -> y, bounced via SBUF),
                # interleaved: chunk k is final once block 24*(k+1) has
                # been written, so it overlaps the remaining compute
                # instead of trailing the last block.
                CH = 12000
                if b >= 25 and (b - 25) % 24 == 0 and (k := (b - 25) // 24) < 3:
                    yb = ypool.tile([IPC, CH], f32r)
                    nc.scalar.dma_start(
                        yb[:, :],
                        h_d[IPC * N:M_OUT, PAD + k * CH:PAD + (k + 1) * CH])
                    nc.scalar.dma_start(y_d[:, k * CH:(k + 1) * CH], yb[:, :])
                # last two blocks: ship y straight from the SBUF stage so
                # the kernel tail doesn't wait on their DRAM writes
                if b >= NBLK - 2:
                    nc.sync.dma_start(
                        y_d[:, n0:n0 + L], stage[IPC * N:M_OUT, :])

            # remaining y span [3*CH, last two blocks) after the loop;
            # reads only blocks <= NBLK-3 so it overlaps the loop tail
            yb = ypool.tile([IPC, CH], f32r)
            span = IR_LEN - L * 2 - 3 * CH
            nc.scalar.dma_start(
                yb[:, 0:span],
                h_d[IPC * N:M_OUT, PAD + 3 * CH:PAD + 3 * CH + span])
            nc.scalar.dma_start(y_d[:, 3 * CH:3 * CH + span], yb[:, 0:span])
    nc.compile()
    _BUILT = nc
    return nc


def _pmask():
    pl = _patch_list()
    pm = np.zeros((IPC * N, len(pl)), np.uint8)
    for k, (i, _, _, _, _) in enumerate(pl):
        pm[4 * i:4 * i + 4, k] = 1
    return pm


def kernel(x, WA, bA, WB, bB, WC, bC):
    from concourse import bass_utils

    A_g, Bv, Cv = _prologue(x, WA, bA, WB, bB, WC, bC)
    offs = _offsets()
    pm = _pmask()
    in_maps = []
    for k in range(NCORES):
        sl = slice(k * IPC, (k + 1) * IPC)
        lhsT, bv = _core_inputs(A_g[sl], Bv[sl], Cv[sl])
        m = {"lhsT": lhsT, "bv": bv, "offs": offs}
        if pm.shape[1]:
            m["pmask"] = pm
        in_maps.append(m)

    nc = _build()
    res = bass_utils.run_bass_kernel_spmd(nc, in_maps, core_ids=list(range(NCORES)))
    y = np.concatenate([res.results[k]["y"] for k in range(NCORES)], axis=0)
    return y[:, None, :].astype(np.float32)



# revision 4
# speedup vs baseline: 1.1489x; 1.1489x over previous
"""DiffFDN Trainium2 kernel, v4: L=250 blocks with 4-deep DMA pipeline.

Per core (4 items): the 48000-step FDN scan becomes 188 blocks of
[64x68]^T @ [64x256] (float32r). History lives in DRAM as one
[68, TPAD] tensor (rows 0-63 per-(line,item) nxt series, rows 64-67 the
y output series). Per block: one SWDGE *indirect* gather (per-row flat
offsets) assembling the 16 time-shifted line reads in one instruction,
one matmul, one PSUM->SBUF copy, one staged HWDGE write to DRAM.

v4 vs v3: block advance L=250 (was 500) so the store(b) -> gather(b+4)
dependency has 4 blocks of slack instead of 2 -- the ~6.4us
write+gather DMA latency chain amortizes over 4 block slots. The
matmul/gather still span 256 columns (blocks overlap by 6 recomputed
columns) because fp32r runs 1 cycle/row only at >=256 moving columns;
the write stores only the first 250.

The gather's in_ AP is the column-sliced prefix H[:, 0:PAD+250(b-3)]
so the Tile dependency tracker sees the true dependency (the write
from block b-4); offsets are physical flat element indices
r*TPAD + (PAD + n0 - d_i).
"""

import numpy as np

SR = 48000
IR_LEN = 48000
DELAYS = [1009, 1123, 1231, 1321, 1433, 1543, 1657, 1777, 1879, 1987,
          2081, 2179, 2287, 2383, 2503, 2617]
N = 16
FEAT = 256
BATCH = 32
NCORES = 8
IPC = BATCH // NCORES
L = 250                        # block advance (store granularity)
NMM = 256                      # matmul/gather column span (>=256 for f32r rate)
PAD = 2620                     # zero padding before t=0 (>= max delay)
TPAD = PAD + IR_LEN + NMM
NBLK = IR_LEN // L             # 192; blocks 0-3 all-zero, skipped
B0 = 4                         # first computed block
M_OUT = IPC * N + IPC          # 68

_BUILT = None


def _expm64(M):
    M = M.astype(np.float64)
    nrm = np.linalg.norm(M, ord=np.inf)
    k = max(0, int(np.ceil(np.log2(max(nrm, 1e-30)))) + 2)
    Ms = M / (2.0 ** k)
    E = np.eye(M.shape[0]) + Ms
    term = Ms.copy()
    for i in range(2, 18):
        term = term @ Ms / i
        E = E + term
    for _ in range(k):
        E = E @ E
    return E


def _prologue(x, WA, bA, WB, bB, WC, bC):
    x = np.asarray(x, np.float32)
    feat = x.mean(axis=1)
    A = np.tanh(feat @ np.asarray(WA).T + bA).reshape(-1, N, N)
    Bv = np.tanh(feat @ np.asarray(WB).T + bB)
    Cv = np.tanh(feat @ np.asarray(WC).T + bC)
    S = np.triu(A, 1)
    S = S - np.swapaxes(S, -1, -2)
    g = 10.0 ** (-3.0 / SR)
    G = g ** np.asarray(DELAYS, np.float64)
    A_g = np.stack([_expm64(S[b]) for b in range(S.shape[0])])
    A_g = (A_g * G[None, None, :]).astype(np.float32)
    return A_g, Bv.astype(np.float32), Cv.astype(np.float32)


def _core_inputs(A_g4, Bv4, Cv4):
    lhsT = np.zeros((IPC * N, M_OUT), np.float32)
    bv = np.zeros((IPC * N, 1), np.float32)
    for j in range(IPC):
        for i in range(N):
            r = 4 * i + j
            for ip in range(N):
                lhsT[r, 4 * ip + j] = A_g4[j, ip, i]
            lhsT[r, IPC * N + j] = Cv4[j, i]
            bv[r, 0] = Bv4[j, i]
    return lhsT, bv


def _offsets():
    """offs[r, b-B0] = flat physical gather offset for row r, block b."""
    offs = np.zeros((IPC * N, NBLK - B0), np.uint32)
    for b in range(B0, NBLK):
        n0 = L * b
        for i in range(N):
            for j in range(IPC):
                r = 4 * i + j
                offs[r, b - B0] = r * TPAD + (PAD + n0 - DELAYS[i])
    return offs


def _build():
    global _BUILT
    if _BUILT is not None:
        return _BUILT
    import concourse.bacc as bacc
    import concourse.bass as bass
    import concourse.mybir as mybir
    import concourse.tile as tile

    fp32 = mybir.dt.float32
    f32r = mybir.dt.float32r
    u32 = mybir.dt.uint32
    nc = bacc.Bacc("TRN2", target_bir_lowering=False, debug=False)
    lhsT_d = nc.dram_tensor("lhsT", [IPC * N, M_OUT], f32r, kind="ExternalInput")
    bv_d = nc.dram_tensor("bv", [IPC * N, 1], f32r, kind="ExternalInput")
    offs_d = nc.dram_tensor("offs", [IPC * N, NBLK - B0], u32, kind="ExternalInput")
    y_d = nc.dram_tensor("y", [IPC, IR_LEN], f32r, kind="ExternalOutput")
    h_d = nc.dram_tensor("hist", [M_OUT, TPAD], f32r)

    ZC = PAD + B0 * L          # zero-init history columns [0, ZC)

    with tile.TileContext(nc) as tc:
        with tc.tile_pool(name="const", bufs=1) as cpool, \
             tc.tile_pool(name="init", bufs=1) as ipool, \
             tc.tile_pool(name="sg", bufs=6) as spool, \
             tc.tile_pool(name="st", bufs=6) as tpool, \
             tc.tile_pool(name="ps", bufs=6, space="PSUM") as ppool, \
             tc.tile_pool(name="yb", bufs=2) as ypool:
            lhsT = cpool.tile([IPC * N, M_OUT], f32r)
            nc.sync.dma_start(lhsT[:, :], lhsT_d[:, :])
            offs = cpool.tile([IPC * N, NBLK - B0], u32)
            nc.sync.dma_start(offs[:, :], offs_d[:, :])

            # zero-init history cols [0, ZC) incl. y rows; Bv impulse
            # lands at col PAD (time 0) via the same staged image.
            z = ipool.tile([M_OUT, ZC], fp32)
            half = ZC // 2
            nc.vector.memset(z[:, 0:half], 0.0)
            nc.gpsimd.memset(z[:, half:], 0.0)
            nc.sync.dma_start(z[0:IPC * N, PAD:PAD + 1].bitcast(f32r), bv_d[:, :])
            nc.scalar.dma_start(h_d[:, 0:ZC].bitcast(fp32), z[:, :])

            CH = 12000         # y bounce chunk (48 blocks)
            CB = CH // L
            for b in range(B0, NBLK):
                n0 = L * b
                Xb = PAD + L * (b - 3)
                S = spool.tile([IPC * N, NMM], f32r)
                nc.gpsimd.indirect_dma_start(
                    out=S[:, :], out_offset=None,
                    in_=h_d[0:IPC * N, 0:Xb],
                    in_offset=bass.IndirectOffsetOnAxis(
                        ap=offs[:, b - B0:b - B0 + 1], axis=1),
                )
                ps = ppool.tile([M_OUT, NMM], fp32)
                nc.tensor.matmul(ps[:, :], lhsT[:, :], S[:, :],
                                 start=True, stop=True)
                stage = tpool.tile([M_OUT, L], f32r)
                nc.vector.tensor_copy(stage[:, :], ps[:, 0:L])
                nc.sync.dma_start(h_d[:, PAD + n0:PAD + n0 + L], stage[:, :])

                # y extraction (hist rows 64..67 -> y, bounced via SBUF),
                # interleaved: chunk k is final once block CB*(k+1)-1 has
                # been written, so it overlaps the remaining compute.
                if b >= CB + 3 and (b - CB - 3) % CB == 0 \
                        and (k := (b - CB - 3) // CB) < 3:
                    yb = ypool.tile([IPC, CH], f32r)
                    nc.scalar.dma_start(
                        yb[:, :],
                        h_d[IPC * N:M_OUT, PAD + k * CH:PAD + (k + 1) * CH])
                    nc.scalar.dma_start(y_d[:, k * CH:(k + 1) * CH], yb[:, :])
                # last two blocks: ship y straight from the SBUF stage so
                # the kernel tail doesn't wait on their DRAM writes
                if b >= NBLK - 2:
                    nc.sync.dma_start(
                        y_d[:, n0:n0 + L], stage[IPC * N:M_OUT, :])

            # remaining y span [3*CH, last two blocks) after the loop;
            # reads only blocks <= NBLK-3 so it overlaps the loop tail
            yb = ypool.tile([IPC, CH], f32r)
            span = IR_LEN - L * 2 - 3 * CH
            nc.scalar.dma_start(
                yb[:, 0:span],
                h_d[IPC * N:M_OUT, PAD + 3 * CH:PAD + 3 * CH + span])
            nc.scalar.dma_start(y_d[:, 3 * CH:3 * CH + span], yb[:, 0:span])
    nc.compile()
    _BUILT = nc
    return nc


def kernel(x, WA, bA, WB, bB, WC, bC):
    from concourse import bass_utils

    A_g, Bv, Cv = _prologue(x, WA, bA, WB, bB, WC, bC)
    offs = _offsets()
    in_maps = []
    for k in range(NCORES):
        sl = slice(k * IPC, (k + 1) * IPC)
        lhsT, bv = _core_inputs(A_g[sl], Bv[sl], Cv[sl])
        in_maps.append({"lhsT": lhsT, "bv": bv, "offs": offs})

    nc = _build()
    res = bass_utils.run_bass_kernel_spmd(nc, in_maps, core_ids=list(range(NCORES)))
    y = np.concatenate([res.results[k]["y"] for k in range(NCORES)], axis=0)
    return y[:, None, :].astype(np.float32)


# revision 6
# speedup vs baseline: 1.2938x; 1.1261x over previous
"""DiffFDN Trainium2 kernel, v5: L=250 blocks, 5-deep DMA pipeline with
stage-patched racing gathers.

Per core (4 items): the 48000-step FDN scan becomes 188 blocks of
[72x72]^T @ [72x256] (float32r). History lives in DRAM as one
[72, TPAD] tensor; row layout puts the three shortest delay lines at
partition bases 0/32/64 (engines want 32-aligned partition bases), the
other lines at 4-row slots in between, and the y output series at rows
68-71.

Per block b: one SWDGE *indirect* gather (per-row flat offsets) that
waits only on the write from block b-5; the columns sourced from block
b-4 (lines 0-2 only) race with that write and are patched afterwards
from block b-4's SBUF stage via three 4-row 32-aligned copies. Then one
matmul, one PSUM->SBUF copy, one HWDGE write to DRAM. The gather/matmul
span 256 columns (>=256 keeps fp32r at 1 cycle/row; blocks overlap by 6
recomputed columns), the store advances 250.

The gather's in_ AP is the column-sliced prefix H[:, 0:PAD+250(b-4)] so
the Tile dependency tracker sees only the write from block b-5; offsets
are physical flat element indices row*TPAD + (PAD + n0 - d_i).
"""

import numpy as np

SR = 48000
IR_LEN = 48000
DELAYS = [1009, 1123, 1231, 1321, 1433, 1543, 1657, 1777, 1879, 1987,
          2081, 2179, 2287, 2383, 2503, 2617]
N = 16
FEAT = 256
BATCH = 32
NCORES = 8
IPC = BATCH // NCORES
L = 250                        # block advance (store granularity)
NMM = 256                      # matmul/gather column span (>=256 for f32r rate)
PAD = 2620                     # zero padding before t=0 (>= max delay)
TPAD = PAD + IR_LEN + NMM
NBLK = IR_LEN // L             # 192; blocks 0-3 all-zero, skipped
B0 = 4                         # first computed block
KD = 5                         # gather(b) waits write(b-KD); block b-4 patched
M_OUT = 72                     # 64 nxt rows (padded layout) + 4 y rows
YR = 68                        # y rows at 68..71

# partition base of each line's 4 item rows: patched lines 0,1,2 at the
# 32-aligned bases so their patch copies are legal engine ops
ROWB = {0: 0, 1: 32, 2: 64, 3: 4, 4: 8, 5: 12, 6: 16, 7: 20, 8: 24,
        9: 28, 10: 36, 11: 40, 12: 44, 13: 48, 14: 52, 15: 56}
# (base, c0): S[base:base+4, c0:NMM] is sourced from block b-4 and patched
PATCHES = [(ROWB[i], DELAYS[i] - (KD - 1) * L) for i in range(N)
           if DELAYS[i] - (KD - 1) * L < NMM]

_BUILT = None


def _expm64(M):
    M = M.astype(np.float64)
    nrm = np.linalg.norm(M, ord=np.inf)
    k = max(0, int(np.ceil(np.log2(max(nrm, 1e-30)))) + 2)
    Ms = M / (2.0 ** k)
    E = np.eye(M.shape[0]) + Ms
    term = Ms.copy()
    for i in range(2, 18):
        term = term @ Ms / i
        E = E + term
    for _ in range(k):
        E = E @ E
    return E


def _prologue(x, WA, bA, WB, bB, WC, bC):
    x = np.asarray(x, np.float32)
    feat = x.mean(axis=1)
    A = np.tanh(feat @ np.asarray(WA).T + bA).reshape(-1, N, N)
    Bv = np.tanh(feat @ np.asarray(WB).T + bB)
    Cv = np.tanh(feat @ np.asarray(WC).T + bC)
    S = np.triu(A, 1)
    S = S - np.swapaxes(S, -1, -2)
    g = 10.0 ** (-3.0 / SR)
    G = g ** np.asarray(DELAYS, np.float64)
    A_g = np.stack([_expm64(S[b]) for b in range(S.shape[0])])
    A_g = (A_g * G[None, None, :]).astype(np.float32)
    return A_g, Bv.astype(np.float32), Cv.astype(np.float32)


def _core_inputs(A_g4, Bv4, Cv4):
    lhsT = np.zeros((M_OUT, M_OUT), np.float32)
    bv = np.zeros((M_OUT, 1), np.float32)
    for j in range(IPC):
        for i in range(N):
            r = ROWB[i] + j
            for ip in range(N):
                lhsT[r, ROWB[ip] + j] = A_g4[j, ip, i]
            lhsT[r, YR + j] = Cv4[j, i]
            bv[r, 0] = Bv4[j, i]
    return lhsT, bv


def _offsets():
    """offs[r, b-B0] = flat physical gather offset for row r, block b."""
    offs = np.zeros((M_OUT, NBLK - B0), np.uint32)
    for b in range(B0, NBLK):
        n0 = L * b
        for i in range(N):
            for j in range(IPC):
                r = ROWB[i] + j
                offs[r, b - B0] = r * TPAD + (PAD + n0 - DELAYS[i])
    return offs


def _build():
    global _BUILT
    if _BUILT is not None:
        return _BUILT
    import concourse.bacc as bacc
    import concourse.bass as bass
    import concourse.mybir as mybir
    import concourse.tile as tile

    fp32 = mybir.dt.float32
    f32r = mybir.dt.float32r
    u32 = mybir.dt.uint32
    nc = bacc.Bacc("TRN2", target_bir_lowering=False, debug=False)
    lhsT_d = nc.dram_tensor("lhsT", [M_OUT, M_OUT], f32r, kind="ExternalInput")
    bv_d = nc.dram_tensor("bv", [M_OUT, 1], f32r, kind="ExternalInput")
    offs_d = nc.dram_tensor("offs", [M_OUT, NBLK - B0], u32, kind="ExternalInput")
    y_d = nc.dram_tensor("y", [IPC, IR_LEN], f32r, kind="ExternalOutput")
    h_d = nc.dram_tensor("hist", [M_OUT, TPAD], f32r)

    ZC = PAD + B0 * L          # zero-init history columns [0, ZC)

    with tile.TileContext(nc) as tc:
        with tc.tile_pool(name="const", bufs=1) as cpool, \
             tc.tile_pool(name="init", bufs=1) as ipool, \
             tc.tile_pool(name="sg", bufs=6) as spool, \
             tc.tile_pool(name="st", bufs=8) as tpool, \
             tc.tile_pool(name="ps", bufs=6, space="PSUM") as ppool, \
             tc.tile_pool(name="yb", bufs=2) as ypool:
            lhsT = cpool.tile([M_OUT, M_OUT], f32r)
            nc.sync.dma_start(lhsT[:, :], lhsT_d[:, :])
            offs = cpool.tile([M_OUT, NBLK - B0], u32)
            nc.sync.dma_start(offs[:, :], offs_d[:, :])

            # zero-init history cols [0, ZC) incl. y rows; Bv impulse
            # lands at col PAD (time 0) via the same staged image.
            z = ipool.tile([M_OUT, ZC], fp32)
            half = ZC // 2
            nc.vector.memset(z[:, 0:half], 0.0)
            nc.gpsimd.memset(z[:, half:], 0.0)
            nc.sync.dma_start(z[:, PAD:PAD + 1].bitcast(f32r), bv_d[:, :])
            nc.scalar.dma_start(h_d[:, 0:ZC].bitcast(fp32), z[:, :])

            CH = 12000         # y bounce chunk (48 blocks)
            CB = CH // L
            # virtual stages for the all-zero blocks 0..3 (impulse at PAD)
            stages = {m: (z, PAD + L * m) for m in range(B0)}
            for b in range(B0, NBLK):
                n0 = L * b
                Xb = PAD + L * (b - KD + 1)
                S = spool.tile([M_OUT, NMM], f32r)
                nc.gpsimd.indirect_dma_start(
                    out=S[:, :], out_offset=None,
                    in_=h_d[0:M_OUT, 0:Xb],
                    in_offset=bass.IndirectOffsetOnAxis(
                        ap=offs[:, b - B0:b - B0 + 1], axis=1),
                )
                # patch the raced columns (sourced from block b-4, whose
                # DRAM write the gather did not wait for) from that
                # block's SBUF stage; 4-row copies at 32-aligned bases
                ptile, pcol = stages[b - KD + 1]
                for base, c0 in PATCHES:
                    ln = NMM - c0
                    src = ptile[base:base + 4, pcol:pcol + ln]
                    if src.dtype != f32r:
                        src = src.bitcast(f32r)
                    nc.vector.tensor_copy(S[base:base + 4, c0:NMM], src)
                ps = ppool.tile([M_OUT, NMM], fp32)
                nc.tensor.matmul(ps[:, :], lhsT[:, :], S[:, :],
                                 start=True, stop=True)
                stage = tpool.tile([M_OUT, L], f32r)
                stages[b] = (stage, 0)
                nc.vector.tensor_copy(stage[:, :], ps[:, 0:L])
                nc.sync.dma_start(h_d[:, PAD + n0:PAD + n0 + L], stage[:, :])

                # y extraction (hist rows 68..71 -> y, bounced via SBUF),
                # interleaved: chunk k is final once block CB*(k+1)-1 has
                # been written, so it overlaps the remaining compute.
                if b >= CB + 3 and (b - CB - 3) % CB == 0 \
                        and (k := (b - CB - 3) // CB) < 3:
                    yb = ypool.tile([IPC, CH], f32r)
                    nc.scalar.dma_start(
                        yb[:, :],
                        h_d[YR:M_OUT, PAD + k * CH:PAD + (k + 1) * CH])
                    nc.scalar.dma_start(y_d[:, k * CH:(k + 1) * CH], yb[:, :])
                # last two blocks: ship y straight from the SBUF stage so
                # the kernel tail doesn't wait on their DRAM writes
                if b >= NBLK - 2:
                    nc.sync.dma_start(
                        y_d[:, n0:n0 + L], stage[YR:M_OUT, :])

            # remaining y span [3*CH, last two blocks) after the loop;
            # reads only blocks <= NBLK-3 so it overlaps the loop tail
            yb = ypool.tile([IPC, CH], f32r)
            span = IR_LEN - L * 2 - 3 * CH
            nc.scalar.dma_start(
                yb[:, 0:span],
                h_d[YR:M_OUT, PAD + 3 * CH:PAD + 3 * CH + span])
            nc.scalar.dma_start(y_d[:, 3 * CH:3 * CH + span], yb[:, 0:span])
    nc.compile()
    _BUILT = nc
    return nc


def kernel(x, WA, bA, WB, bB, WC, bC):
    from concourse import bass_utils

    A_g, Bv, Cv = _prologue(x, WA, bA, WB, bB, WC, bC)
    offs = _offsets()
    in_maps = []
    for k in range(NCORES):
        sl = slice(k * IPC, (k + 1) * IPC)
        lhsT, bv = _core_inputs(A_g[sl], Bv[sl], Cv[sl])
        in_maps.append({"lhsT": lhsT, "bv": bv, "offs": offs})

    nc = _build()
    res = bass_utils.run_bass_kernel_spmd(nc, in_maps, core_ids=list(range(NCORES)))
    y = np.concatenate([res.results[k]["y"] for k in range(NCORES)], axis=0)
    return y[:, None, :].astype(np.float32)
